# revision 3
# baseline (speedup 1.0000x reference)
"""Trainium2 Bass kernel: MLA attention + top-2 MoE (8 experts), fp8 DoubleRow.

Sharding (8 NeuronCores):
  Launch 1 (attention): core c = (batch b=c//4, head-group g=c%4 of 4 heads).
    Host does LN1 exactly and ships h^T in fp8 K-major layout; Wdkv@Wukv is
    fused host-side into a single kv projection weight. Device computes
    qT/kvT (transposed, raw-scaled bf16), kv natural (kva, + ones column for
    softmax denominators), causal scores in 128x128 blocks, exp on ACT with
    the dequant scale folded in, post-exp triangular 0/1 mask multiply on
    GPSIMD, attnV with denominator accumulation, per-query rescale to fp8,
    PE transposes, and out-projection via fp8 DoubleRow. xo^T partials in
    bf16; host sums them.
  Host: xnew = x + sum(partials); LN2; gate logits; top-2 softmax; gather.
  Launch 2 (expert MLP): core e = expert e; both matmuls fp8 DoubleRow;
    gelu on ACT with per-partition bias + dequant scale; combine weights,
    b2, and output dequant applied on host during scatter-add.
"""

import numpy as np
import ml_dtypes

import concourse.bass as bass
import concourse.bacc as bacc
import concourse.mybir as mybir
from concourse.tile import TileContext
from concourse.masks import make_identity
from concourse.bass_utils import run_bass_kernel_spmd

F32 = mybir.dt.float32
BF16 = mybir.dt.bfloat16
FP8 = mybir.dt.float8e4
AF = mybir.ActivationFunctionType
DR = mybir.MatmulPerfMode.DoubleRow
ALU = mybir.AluOpType

E4 = ml_dtypes.float8_e4m3
BF = ml_dtypes.bfloat16

B, S, D = 2, 2048, 1024
H, DH, DL = 16, 64, 512
E, DFF, TOPK = 8, 2048, 2
HC = 4              # heads per core
HDC = HC * DH       # 256 attn dims per core
EPS = 1e-5
SA = 8.0            # attn fp8 pre-quant scale (compile-time constant)
SQ8 = 4.0           # qT fp8 pre-quant scale
SKV8 = 4.0          # kvT fp8 pre-quant scale
NQT = S // 128      # 16 query tiles of 128

_cache = {}


def build_l1():
    nc = bacc.Bacc()
    hT8 = nc.dram_tensor("hT8", [128, 8, S], FP8, kind="ExternalInput")
    wq8 = nc.dram_tensor("wq8", [128, 8, 256], FP8, kind="ExternalInput")
    wkv8 = nc.dram_tensor("wkv8", [128, 8, 256], FP8, kind="ExternalInput")
    wo8 = nc.dram_tensor("wo8", [128, 2, D], FP8, kind="ExternalInput")
    tri = nc.dram_tensor("tri", [128, 128], BF16, kind="ExternalInput")
    scl = nc.dram_tensor("scl", [128, 4], F32, kind="ExternalInput")
    xoT = nc.dram_tensor("xoT", [128, 8, S], BF16, kind="ExternalOutput")

    with TileContext(nc) as tc:
        import contextlib
        with contextlib.ExitStack() as ctx:
            singles = ctx.enter_context(tc.tile_pool(name="singles", bufs=1))
            big = ctx.enter_context(tc.tile_pool(name="big", bufs=1))
            work = ctx.enter_context(tc.tile_pool(name="work", bufs=10))
            psA = ctx.enter_context(tc.tile_pool(name="psA", bufs=2, space="PSUM"))
            psS = ctx.enter_context(tc.tile_pool(name="psS", bufs=3, space="PSUM"))

            tri_sb = singles.tile([128, 128], BF16, name="tri_sb", tag="tri_sb")
            nc.gpsimd.dma_start(out=tri_sb, in_=tri[:, :])
            scl_sb = singles.tile([128, 4], F32, name="scl_sb", tag="scl_sb")
            nc.gpsimd.dma_start(out=scl_sb, in_=scl[:, :])
            ident = singles.tile([128, 128], BF16, name="ident", tag="ident")
            make_identity(nc, ident)

            # critical-path DMA order: weights, then hT slices, wo last
            wq_sb = singles.tile([128, 8, 256], FP8, name="wq_sb", tag="wq_sb")
            nc.sync.dma_start(out=wq_sb, in_=wq8[:, :, :])
            wkv_sb = singles.tile([128, 8, 256], FP8, name="wkv_sb", tag="wkv_sb")
            nc.scalar.dma_start(out=wkv_sb, in_=wkv8[:, :, :])
            h_sb = big.tile([128, 8, S], FP8, name="h_sb", tag="h_sb")
            # chunk-0 columns of every k-slice first: unblocks proj_chunk(0)
            for t in range(4):
                eng = nc.sync if t % 2 == 0 else nc.scalar
                eng.dma_start(out=h_sb[:, 2 * t:2 * t + 2, 0:512],
                              in_=hT8[:, 2 * t:2 * t + 2, 0:512])
            for t in range(4):
                eng = nc.sync if t % 2 == 0 else nc.scalar
                eng.dma_start(out=h_sb[:, 2 * t:2 * t + 2, 512:S],
                              in_=hT8[:, 2 * t:2 * t + 2, 512:S])
            wo_sb = singles.tile([128, 2, D], FP8, name="wo_sb", tag="wo_sb")
            nc.gpsimd.dma_start(out=wo_sb, in_=wo8[:, :, :])

            # --- projections (emitted chunk-wise, interleaved with attention) ---
            # qT/kvT tile layout [128, 2, S]: slot tI covers heads (2tI, 2tI+1)
            # with partitions 0:64 = head 2tI dims, 64:128 = head 2tI+1 dims.
            qT = big.tile([128, 2, S], FP8, name="qT", tag="qT")
            kvT = big.tile([128, 2, S], FP8, name="kvT", tag="kvT")
            kva = [big.tile([128, HC, DH + 1], BF16, name=f"kva{kt}",
                            tag=f"kva{kt}") for kt in range(NQT)]

            def _proj_qkv(dst, w_sb, hf, c, sc_col, on_act=False):
                ps = psA.tile([128, 512], F32, name="psA", tag="psA")
                for half in range(2):       # DR moving chunks of 256
                    sl = slice(c * 512 + half * 256,
                               c * 512 + half * 256 + 256)
                    for t in range(4):
                        nc.tensor.matmul(
                            ps[:, half * 256:half * 256 + 256],
                            w_sb[:, 2 * t:2 * t + 2, hf * 128:hf * 128 + 128],
                            h_sb[:, 2 * t:2 * t + 2, sl],
                            start=(t == 0), stop=(t == 3), perf_mode=DR)
                if on_act:
                    nc.scalar.activation(
                        out=dst[:, hf, c * 512:(c + 1) * 512], in_=ps,
                        func=AF.Copy, scale=scl_sb[:, sc_col:sc_col + 1])
                else:
                    nc.vector.tensor_scalar_mul(
                        out=dst[:, hf, c * 512:(c + 1) * 512], in0=ps,
                        scalar1=scl_sb[:, sc_col:sc_col + 1])

            def _proj_kva(ktp):
                ps = psA.tile([128, 512], F32, name="psA", tag="psA")
                for j in range(2):
                    kt = 2 * ktp + j
                    for t in range(4):
                        nc.tensor.matmul(
                            ps[:, j * 256:j * 256 + 256],
                            h_sb[:, 2 * t:2 * t + 2, kt * 128:kt * 128 + 128],
                            wkv_sb[:, 2 * t:2 * t + 2, :],
                            start=(t == 0), stop=(t == 3), perf_mode=DR)
                for j in range(2):
                    kt = 2 * ktp + j
                    nc.vector.tensor_scalar_mul(
                        out=kva[kt][:, :, 0:DH].rearrange(
                            "p h (hf d) -> p h hf d", hf=2),
                        in0=ps[:, j * 256:j * 256 + 256].rearrange(
                            "p (hf h d) -> p h hf d", hf=2, h=HC),
                        scalar1=scl_sb[:, 1:2])
                    nc.vector.memset(kva[kt][:, :, DH:DH + 1], 1.0)

            def proj_pieces(c):
                """qT/kvT/kva for S-slice [512c, 512c+512) as 6 closures.
                Both halves of kvT/qT are needed before any score matmul."""
                return [
                    lambda: _proj_qkv(kvT, wkv_sb, 0, c, 3),
                    lambda: _proj_qkv(kvT, wkv_sb, 1, c, 3),
                    lambda: _proj_qkv(qT, wq_sb, 0, c, 0),
                    lambda: _proj_qkv(qT, wq_sb, 1, c, 0),
                    lambda: _proj_kva(2 * c),
                    lambda: _proj_kva(2 * c + 1),
                ]

            # --- causal attention ---
            attn_sb = [big.tile([128, HDC], BF16, name=f"attn{qp}",
                                tag=f"attn{qp}") for qp in range(NQT)]
            attnT = big.tile([128, 2, S], FP8, name="attnT", tag="attnT")
            xoT_sb = big.tile([128, 8, S], BF16, name="xoT_sb", tag="xoT_sb")

            def outproj_unit(cp, dt, split_act):
                """One xoT [128,512] tile = Wo_g^T @ attnT, fp8 DR, dequant."""
                ps = psA.tile([128, 512], F32, name="psA", tag="psA")
                sl = slice(cp * 512, cp * 512 + 512)
                for half in range(2):
                    s2 = slice(cp * 512 + half * 256, cp * 512 + half * 256 + 256)
                    nc.tensor.matmul(
                        ps[:, half * 256:half * 256 + 256],
                        wo_sb[:, 0:2, dt * 128:dt * 128 + 128],
                        attnT[:, 0:2, s2],
                        start=True, stop=True, perf_mode=DR)
                if split_act and dt % 2 == 1:
                    # tail: ACT is idle, split evacs with DVE
                    nc.scalar.activation(
                        out=xoT_sb[:, dt, sl],
                        in_=ps, func=AF.Copy, scale=scl_sb[:, 2:3])
                else:
                    nc.vector.tensor_scalar_mul(
                        out=xoT_sb[:, dt, sl],
                        in0=ps, scalar1=scl_sb[:, 2:3])
                if dt == 3 and cp == 3:
                    nc.sync.dma_start(out=xoT[:, 0:4, sl],
                                      in_=xoT_sb[:, 0:4, sl])
                elif dt == 7:
                    # dependent out-DMAs go on the idle SP queue only: an
                    # engine-issued DMA blocks that engine's sequencer while
                    # its input deps are pending.
                    if cp == 3:
                        nc.sync.dma_start(out=xoT[:, 4:8, sl],
                                          in_=xoT_sb[:, 4:8, sl])
                    else:
                        nc.sync.dma_start(out=xoT[:, :, sl],
                                          in_=xoT_sb[:, :, sl])

            pbTs = {}

            def score_head(qp, h):
                """scores + exp + diag-mask for one head of query tile qp."""
                nkt = qp + 1
                if h == 0:
                    pbTs[qp] = []
                pO = 32 * h
                pbT = work.tile([128, S], BF16, name="pbT", tag="pbT")
                pbTs[qp].append(pbT)
                for g0 in range(0, nkt, 8):
                    gn = min(8, nkt - g0)
                    ps = psS.tile([128, 1024], F32, name="psS", tag="psS")
                    for j in range(gn):
                        kt = g0 + j
                        nc.tensor.matmul(
                            ps[:, j * 128:j * 128 + 128],
                            kvT[pO:pO + 32, 0:2, kt * 128:kt * 128 + 128],
                            qT[pO:pO + 32, 0:2, qp * 128:qp * 128 + 128],
                            start=True, stop=True, perf_mode=DR,
                            tile_position=(pO, 0))
                    nc.scalar.activation(
                        out=pbT[:, g0 * 128:(g0 + gn) * 128],
                        in_=ps[:, 0:gn * 128], func=AF.Exp,
                        scale=1.0 / (SQ8 * SKV8 * (DH ** 0.5)))
                # triangular mask on the diagonal block (kt == qp)
                nc.gpsimd.tensor_mul(
                    out=pbT[:, qp * 128:qp * 128 + 128],
                    in0=pbT[:, qp * 128:qp * 128 + 128], in1=tri_sb)

            P2s = {}

            def attnv_half(qp, half):
                nkt = qp + 1
                if half == 0:
                    P2s[qp] = psA.tile([128, HC, DH + 1], F32, name="P2",
                                       tag="psA")
                P2 = P2s[qp]
                for h in (0, 1) if half == 0 else (2, 3):
                    for kt in range(nkt):
                        nc.tensor.matmul(
                            P2[:, h, :], pbTs[qp][h][:, kt * 128:kt * 128 + 128],
                            kva[kt][:, h, :],
                            start=(kt == 0), stop=(kt == nkt - 1))
                if half == 1:
                    del pbTs[qp]
                    del P2s[qp]
                    rec = work.tile([128, HC], F32, name="rec", tag="rec")
                    nc.vector.reciprocal(out=rec, in_=P2[:, :, DH:DH + 1])
                    for h in range(HC):
                        nc.vector.tensor_scalar(
                            out=attn_sb[qp][:, h * DH:(h + 1) * DH],
                            in0=P2[:, h, 0:DH], scalar1=rec[:, h:h + 1],
                            scalar2=SA, op0=ALU.mult, op1=ALU.mult)

            op_queue = []

            def trans_round(qp):
                for i in range(2):
                    pt = psA.tile([128, 128], BF16, name="ptT", tag="psA")
                    nc.tensor.transpose(pt, attn_sb[qp][:, i * 128:(i + 1) * 128],
                                        ident)
                    nc.vector.tensor_copy(
                        out=attnT[:, i, qp * 128:(qp + 1) * 128], in_=pt)
                if qp % 4 == 3:
                    op_queue.extend((qp // 4, dt) for dt in range(8))

            # 3-stage software pipeline over query tiles. Projection pieces
            # for chunk c+1 are woven between score heads of chunk c's last
            # two rounds; attnV halves woven between score heads keep ACT fed.
            # Prologue: weave chunk-0 pieces with qp=0's heads so the first
            # exp starts as early as possible.
            # prologue: all qT/kvT halves before any score; ACT is idle
            # here so alternate the evacuations between DVE and ACT
            _proj_qkv(kvT, wkv_sb, 0, 0, 3, on_act=False)
            _proj_qkv(kvT, wkv_sb, 1, 0, 3, on_act=False)
            _proj_qkv(qT, wq_sb, 0, 0, 0, on_act=False)
            _proj_qkv(qT, wq_sb, 1, 0, 0, on_act=False)
            pp = proj_pieces(0)
            score_head(0, 0)
            score_head(0, 1)
            pp[4]()
            score_head(0, 2)
            score_head(0, 3)
            pp[5]()
            pending = []
            for c in range(4):
                for qp in range(4 * c, 4 * c + 4):
                    if c == 0 and qp == 0:
                        continue
                    if qp % 4 == 2 and c < 3:
                        pending = proj_pieces(c + 1)
                    for h in range(HC):
                        if pending:
                            pending.pop(0)()
                        score_head(qp, h)
                        if h == 1 and qp >= 1:
                            attnv_half(qp - 1, 0)
                    if qp >= 1:
                        attnv_half(qp - 1, 1)
                    if qp >= 2:
                        trans_round(qp - 2)
                    for _ in range(2):      # spread outproj over qp rounds
                        if op_queue:
                            outproj_unit(*op_queue.pop(0), False)
            attnv_half(NQT - 1, 0)
            attnv_half(NQT - 1, 1)
            trans_round(NQT - 2)
            trans_round(NQT - 1)
            while op_queue:
                c0, dt0 = op_queue.pop(0)
                outproj_unit(c0, dt0, True)
    nc.compile()
    return nc


def build_l2(capT: int):
    nc = bacc.Bacc()
    xe8 = nc.dram_tensor("xe8", [128, 8, capT], FP8, kind="ExternalInput")
    w18 = nc.dram_tensor("w18", [128, 8, DFF], FP8, kind="ExternalInput")
    w28 = nc.dram_tensor("w28", [128, 16, D], FP8, kind="ExternalInput")
    b1t = nc.dram_tensor("b1t", [128, 16], F32, kind="ExternalInput")
    sc2 = nc.dram_tensor("sc2", [128, 1], F32, kind="ExternalInput")
    yT = nc.dram_tensor("yT", [128, 8, capT], BF16, kind="ExternalOutput")

    chunks = []
    off = 0
    while off < capT:
        n = min(512, capT - off)
        chunks.append((off, n))
        off += n

    def halves(off, n):
        o2 = off
        while o2 < off + n:
            n2 = min(256, off + n - o2)
            yield o2, n2
            o2 += n2

    with TileContext(nc) as tc:
        import contextlib
        with contextlib.ExitStack() as ctx:
            singles = ctx.enter_context(tc.tile_pool(name="singles", bufs=1))
            big = ctx.enter_context(tc.tile_pool(name="big", bufs=1))
            psH = ctx.enter_context(tc.tile_pool(name="psH", bufs=3, space="PSUM"))
            psY = ctx.enter_context(tc.tile_pool(name="psY", bufs=3, space="PSUM"))

            # critical-path DMAs: w1 on the two HWDGE queues, xe on the idle
            # Pool software-DGE queue (3rd parallel channel), w2 later
            w1_sb = singles.tile([128, 8, DFF], FP8, name="w1_sb", tag="w1_sb")
            xe_sb = big.tile([128, 8, capT], FP8, name="xe_sb", tag="xe_sb")
            for q in range(4):          # ft-column groups: hid(ft) gated on q
                eng = nc.sync if q % 2 == 0 else nc.scalar
                eng.dma_start(out=w1_sb[:, :, q * 512:(q + 1) * 512],
                              in_=w18[:, :, q * 512:(q + 1) * 512])
            for t in range(4):
                nc.gpsimd.dma_start(out=xe_sb[:, 2 * t:2 * t + 2, :],
                                    in_=xe8[:, 2 * t:2 * t + 2, :])
            b1_sb = singles.tile([128, 16], F32, name="b1_sb", tag="b1_sb")
            nc.sync.dma_start(out=b1_sb, in_=b1t[:, :])
            sc_sb = singles.tile([128, 1], F32, name="sc_sb", tag="sc_sb")
            nc.scalar.dma_start(out=sc_sb, in_=sc2[:, :])
            w2_sb = singles.tile([128, 16, D], FP8, name="w2_sb", tag="w2_sb")
            for q in range(4):          # dt-column groups: y(dt) gated on q
                eng = nc.sync if q % 2 == 0 else nc.scalar
                eng.dma_start(out=w2_sb[:, :, q * 256:(q + 1) * 256],
                              in_=w28[:, :, q * 256:(q + 1) * 256])

            hid8 = big.tile([128, 16, capT], FP8, name="hid8", tag="hid8")
            yT_sb = big.tile([128, 8, capT], BF16, name="yT_sb", tag="yT_sb")

            def hid_chunk(off, n):
                for ft in range(16):
                    ps = psH.tile([128, 512], F32, name="psH", tag="psH")
                    for (o2, n2) in halves(off, n):
                        rel = o2 - off
                        for t in range(4):
                            nc.tensor.matmul(
                                ps[:, rel:rel + n2],
                                w1_sb[:, 2 * t:2 * t + 2,
                                      ft * 128:ft * 128 + 128],
                                xe_sb[:, 2 * t:2 * t + 2, o2:o2 + n2],
                                start=(t == 0), stop=(t == 3), perf_mode=DR)
                    nc.scalar.activation(
                        out=hid8[:, ft, off:off + n], in_=ps[:, 0:n],
                        func=AF.Gelu, bias=b1_sb[:, ft:ft + 1],
                        scale=sc_sb[:, 0:1])

            def y_chunk(off, n, ci, last=False):
                for dt in range(8):
                    ps = psY.tile([128, 512], F32, name="psY", tag="psY")
                    for (o2, n2) in halves(off, n):
                        rel = o2 - off
                        for f in range(8):
                            nc.tensor.matmul(
                                ps[:, rel:rel + n2],
                                w2_sb[:, 2 * f:2 * f + 2,
                                      dt * 128:dt * 128 + 128],
                                hid8[:, 2 * f:2 * f + 2, o2:o2 + n2],
                                start=(f == 0), stop=(f == 7), perf_mode=DR)
                    if last and dt % 2 == 1:
                        nc.scalar.activation(out=yT_sb[:, dt, off:off + n],
                                             in_=ps[:, 0:n], func=AF.Identity)
                    else:
                        nc.vector.tensor_copy(out=yT_sb[:, dt, off:off + n],
                                              in_=ps[:, 0:n])
                nc.sync.dma_start(out=yT[:, :, off:off + n],
                                  in_=yT_sb[:, :, off:off + n])

            # software pipeline: y-stage of chunk i-1 overlaps hid of chunk i
            for ci, (off, n) in enumerate(chunks):
                hid_chunk(off, n)
                if ci > 0:
                    y_chunk(*chunks[ci - 1], ci - 1)
            y_chunk(*chunks[-1], len(chunks) - 1, last=True)
    nc.compile()
    return nc


def _ln(x, g, b):
    mu = x.mean(-1, keepdims=True)
    var = ((x - mu) ** 2).mean(-1, keepdims=True)
    return (x - mu) / np.sqrt(var + EPS) * g + b


def _kmaj(a, kt):
    """[K, N] -> [128, kt, N] with row t*128+p on partition p, slot t."""
    K, N = a.shape
    assert K == kt * 128
    return np.ascontiguousarray(a.reshape(kt, 128, N).transpose(1, 0, 2))


def kernel(x, mask, ln1_scale, ln1_bias, Wq, Wdkv, Wukv, Wo,
           ln2_scale, ln2_bias, Wgate, bgate, We1, be1, We2, be2,
           _collect=None):
    x = np.asarray(x, np.float32)
    h = _ln(x, np.asarray(ln1_scale, np.float32), np.asarray(ln1_bias, np.float32))
    h8 = h.astype(E4)

    Wq_f = np.asarray(Wq, np.float32)
    Wkv = np.asarray(Wdkv, np.float32) @ np.asarray(Wukv, np.float32)
    Wo_f = np.asarray(Wo, np.float32)
    s_q = 192.0 / np.abs(Wq_f).max()
    s_kv = 192.0 / np.abs(Wkv).max()
    s_o = 192.0 / np.abs(Wo_f).max()
    wq8_full = (Wq_f * s_q).astype(E4)
    wkv8_full = (Wkv * s_kv).astype(E4)
    wo8_full = (Wo_f * s_o).astype(E4)

    ii = np.arange(128)
    tri = (ii[:, None] <= ii[None, :]).astype(BF)          # keep key <= query
    scl = np.zeros((128, 4), np.float32)
    scl[:, 0] = SQ8 / s_q            # qT fp8 evac
    scl[:, 1] = 1.0 / s_kv           # kva evac
    scl[:, 2] = 1.0 / (SA * s_o)     # xoT evac
    scl[:, 3] = SKV8 / s_kv          # kvT fp8 evac

    l1_maps = []
    for c in range(8):
        b, g = c // 4, c % 4
        cs = slice(g * HDC, (g + 1) * HDC)
        def _abperm(w):
            # [D, 256] group cols [h, half, d32] -> [half, h, d32]
            return np.ascontiguousarray(
                w.reshape(D, HC, 2, 32).transpose(0, 2, 1, 3).reshape(D, HDC))
        l1_maps.append({
            "hT8": _kmaj(np.ascontiguousarray(h8[b].T), 8),
            "wq8": _kmaj(_abperm(wq8_full[:, cs]), 8),
            "wkv8": _kmaj(_abperm(wkv8_full[:, cs]), 8),
            "wo8": _kmaj(wo8_full[cs, :], 2),
            "tri": tri,
            "scl": scl,
        })

    if "l1" not in _cache:
        _cache["l1"] = build_l1()
    r1 = run_bass_kernel_spmd(_cache["l1"], l1_maps, core_ids=list(range(8)))
    if _collect is not None:
        _collect["r1"] = r1

    xnew = x.copy()
    for c in range(8):
        b = c // 4
        xo = r1.results[c]["xoT"].astype(np.float32)        # [128, 8, S]
        xnew[b] += xo.transpose(1, 0, 2).reshape(D, S).T
    xf = xnew.reshape(B * S, D)

    h2 = _ln(xf, np.asarray(ln2_scale, np.float32),
             np.asarray(ln2_bias, np.float32)).astype(np.float32)
    logits = h2 @ np.asarray(Wgate, np.float32) + np.asarray(bgate, np.float32)
    order = np.argsort(-logits, axis=1, kind="stable")[:, :TOPK]
    tv = np.take_along_axis(logits, order, axis=1)
    ex = np.exp(tv - tv.max(axis=1, keepdims=True))
    wtop = (ex / ex.sum(axis=1, keepdims=True)).astype(np.float32)

    idxs, wts = [], []
    for e in range(E):
        m_e = (order == e)
        rows = np.nonzero(m_e.any(axis=1))[0]
        idxs.append(rows)
        wts.append((wtop * m_e).sum(axis=1)[rows].astype(np.float32))
    maxc = max(len(r) for r in idxs)
    capT = max(512, ((maxc + 127) // 128) * 128)

    h2_8 = h2.astype(E4)
    We1_f, We2_f = np.asarray(We1, np.float32), np.asarray(We2, np.float32)
    be1_f, be2_f = np.asarray(be1, np.float32), np.asarray(be2, np.float32)
    l2_maps, s2s = [], []
    for e in range(E):
        n = len(idxs[e])
        xe = np.zeros((D, capT), E4)
        xe[:, :n] = h2_8[idxs[e]].T
        s1 = 192.0 / np.abs(We1_f[e]).max()
        s2 = 192.0 / np.abs(We2_f[e]).max()
        s2s.append(s2)
        sc2 = np.full((128, 1), 1.0 / s1, np.float32)
        l2_maps.append({
            "xe8": _kmaj(xe, 8),
            "w18": _kmaj((We1_f[e] * s1).astype(E4), 8),
            "w28": _kmaj((We2_f[e] * s2).astype(E4), 16),
            "b1t": np.ascontiguousarray(be1_f[e].reshape(16, 128).T),
            "sc2": sc2,
        })

    key = ("l2", capT)
    if key not in _cache:
        _cache[key] = build_l2(capT)
    r2 = run_bass_kernel_spmd(_cache[key], l2_maps, core_ids=list(range(8)))
    if _collect is not None:
        _collect["r2"] = r2

    out = xf
    for e in range(E):
        n = len(idxs[e])
        y = r2.results[e]["yT"].astype(np.float32)           # [128, 8, capT]
        y = y.transpose(1, 0, 2).reshape(D, capT)[:, :n].T   # [n, D] raw
        out[idxs[e]] += (wts[e] / s2s[e])[:, None] * y + wts[e][:, None] * be2_f[e]
    return out.reshape(B, S, D).astype(np.float32)
